# revision 1
# baseline (speedup 1.0000x reference)
"""Trainium2 Bass kernel for nn_Decoder_58531814310243 (diff-transformer decoder layer).

Computation: h = rmsnorm(x); h = selfdiffattn(h) + h; h = 2*crossdiffattn(h, enc);
h = swiglu(rmsnorm(h)) + h.

Sharding: 8 cores = batch(4) x sequence-half(2), zero collectives. Each core
owns 512 query tokens of one batch and recomputes the cheap full-sequence K/V
projections locally. Activations live in transposed [feature, token] layout so
every matmul contracts over the partition dim. Matmuls run in bf16 (fp32 PSUM);
softmax and norm statistics stay fp32.

Tricks:
  - tokens are permuted per-core so the local 512 tokens sit in columns [0:512)
    (the causal mask input absorbs the permutation);
  - softmax denominators come free from a ones-augmented V column;
  - the diff-attn combine (s1 - lam*s2, then per-head RMSNorm) is algebraically
    rearranged to avoid dividing by softmax denominators elementwise:
        u = O1 - (lam*d1/d2)*O2,   o_norm = u * r,
        r = (1-lam0) * rsqrt(mean_d(u^2) + eps*d1^2)
    where O1/O2 are unnormalized exp@V products and d1/d2 the exp row sums;
  - per-token (free-dim) scalars are broadcast across partitions with tiny PE
    matmuls (pattern^T @ row_vector); per-head stats batch into [16,512] tiles;
  - causal mask is a {0,1} bf16 multiplier applied to exp(scores);
  - rsqrt is computed as exp(-0.5*ln(v)) to stay inside one ACT table set.
"""

import sys

for _p in ("/opt/trn_rl_repo", "/root/.axon_site/_ro/trn_rl_repo"):
    if _p not in sys.path:
        sys.path.insert(0, _p)

import contextlib

import numpy as np
import ml_dtypes

import concourse.bacc as bacc
import concourse.mybir as mybir
import concourse.tile as tile

P = 128
B, T, D, H, HS = 4, 1024, 1024, 16, 64
DFF = 4 * D
TLOC = T // 2
S = T
KT = D // P            # 8 contraction tiles over D
NQC = (H * 2 * HS) // P  # 16 chunks of q/k projection dim
FFC = DFF // P         # 32 chunks of ffn hidden dim
SJ = S // P            # 8 key tiles
EPS = 1e-6
LAM0 = 0.8
SCALE = 1.0 / 8.0      # 1/sqrt(HS)

f32 = mybir.dt.float32
b16 = mybir.dt.bfloat16
AF = mybir.ActivationFunctionType
ALU = mybir.AluOpType
bf = ml_dtypes.bfloat16

N_CORES = 8


# ================================================================= program ==

def _cp(nc, idx, out, in_):
    """Alternate PSUM->SBUF copies between the scalar and vector engines."""
    if idx % 2:
        nc.scalar.copy(out, in_)
    else:
        nc.vector.tensor_copy(out, in_)


def _lam_from_inputs(nc, pool, lq1, lk1, lq2, lk2, name):
    """lam[16,1] = exp(sum(lq1*lk1,-1)) - exp(sum(lq2*lk2,-1)) + LAM0."""
    t = pool.tile([H, HS], f32, tag=f"lamt_{name}", name=f"lamt_{name}")
    s1 = pool.tile([H, 1], f32, tag=f"lams1_{name}", name=f"lams1_{name}")
    s2 = pool.tile([H, 1], f32, tag=f"lams2_{name}", name=f"lams2_{name}")
    lam = pool.tile([H, 1], f32, tag=f"lam_{name}", name=f"lam_{name}")
    nc.vector.tensor_mul(t[:], lq1[:], lk1[:])
    nc.vector.reduce_sum(s1[:], t[:], axis=mybir.AxisListType.X)
    nc.vector.tensor_mul(t[:], lq2[:], lk2[:])
    nc.vector.reduce_sum(s2[:], t[:], axis=mybir.AxisListType.X)
    nc.scalar.activation(s1[:], s1[:], AF.Exp)
    nc.scalar.activation(s2[:], s2[:], AF.Exp)
    nc.vector.tensor_sub(lam[:], s1[:], s2[:])
    nc.vector.tensor_scalar_add(lam[:], lam[:], LAM0)
    return lam


def _rmsnorm(nc, tc, stk, src, g, ones_c, ones_r, out_b16, W, name, psp=None):
    """out_b16[P,KT,W] = bf16( src * g[d] * rsqrt(mean_d(src^2) + EPS) ).

    src: [P,KT,W] fp32 SBUF. Partition-dim sum-of-squares via ones-matmul,
    rsqrt via exp(-0.5*ln), per-token broadcast via ones-row matmul.
    psp: shared PSUM pool with 2KB "pj" slots (or None to create one).
    """
    sqp = stk.enter_context(tc.tile_pool(name=f"rq_{name}", bufs=3))
    stp = stk.enter_context(tc.tile_pool(name=f"rs_{name}", bufs=2))
    ptag = "pj"
    if psp is None:
        psp = stk.enter_context(tc.tile_pool(name=f"rp_{name}", bufs=1, space="PSUM"))
        ptag = "ss"
    for th in range(W // 512):
        sl = slice(512 * th, 512 * (th + 1))
        ssps = psp.tile([1, 512], f32, tag=ptag, name=f"rss_{name}_{th}")
        for kt in range(KT):
            sq = sqp.tile([P, 512], f32, tag="sq", name=f"rsq_{name}_{th}_{kt}")
            nc.vector.tensor_mul(sq[:], src[:, kt, sl], src[:, kt, sl])
            nc.tensor.matmul(ssps[:], ones_c[:], sq[:], start=(kt == 0), stop=(kt == KT - 1))
        v = stp.tile([1, 512], f32, tag="v", name=f"rv_{name}_{th}")
        nc.vector.tensor_scalar(v[:], ssps[:], 1.0 / D, EPS, op0=ALU.mult, op1=ALU.add)
        nc.scalar.activation(v[:], v[:], AF.Ln)
        r = stp.tile([1, 512], f32, tag="r", name=f"rr_{name}_{th}")
        nc.scalar.activation(r[:], v[:], AF.Exp, scale=-0.5)
        rb = psp.tile([P, 512], f32, tag=ptag if ptag == "pj" else "rb",
                      name=f"rrb_{name}_{th}")
        nc.tensor.matmul(rb[:], ones_r[:], r[:], start=True, stop=True)
        for kt in range(KT):
            nc.vector.scalar_tensor_tensor(
                out_b16[:, kt, sl], src[:, kt, sl], g[:, kt : kt + 1], rb[:],
                op0=ALU.mult, op1=ALU.mult)


def _attn(nc, tc, stk, shared, *, q_rhs, kv_rhs, wq, wk, wv, wo, lam, mask,
          patP, patB, resid, out_ap, name):
    """One diff-attention block, transposed layout.

    q_rhs [P,KT,TLOC] b16; kv_rhs [P,KT,S] b16; wq/wk [P,KT,2048], wv/wo
    [P,KT,1024] dram b16. mask [P,SJ,TLOC] b16 {0,1} or None.
    resid: [P,KT,TLOC]-sliceable b16 AP or None.
    out_ap: if resid: bf16 out (attn+resid); else fp32 out (2*attn).
    """
    big = stk.enter_context(tc.tile_pool(name=f"ab_{name}", bufs=1))
    wp, ep, stats, psA, psS, psO = (shared[k] for k in
                                    ("wp", "ep", "stats", "psA", "psS", "psO"))

    QT = big.tile([P, NQC, TLOC], b16, tag="QT", name=f"QT_{name}")
    KTt = big.tile([P, NQC, S], b16, tag="KTt", name=f"KTt_{name}")
    VA = big.tile([P, SJ, H, HS + 1], b16, tag="VA", name=f"VA_{name}")
    O1S = big.tile([P, H // 2, TLOC], f32, tag="O1S", name=f"O1S_{name}")
    O2S = big.tile([P, H // 2, TLOC], f32, tag="O2S", name=f"O2S_{name}")
    # fp32: u = O1 - c*O2 cancels significantly; bf16 spills here cost ~0.5%
    # output error. O1S is overwritten in place by u; o_norm lands in QT
    # (whose scores-phase contents are dead by then).

    # ---- Q^T projection [2048, TLOC]; chunk h rows: q1 in 0:64, q2 in 64:128
    for half in range(2):
        wt = wp.tile([P, KT, 1024], b16, tag="w", name=f"wq_{name}_{half}")
        nc.sync.dma_start(wt[:], wq[:, :, 1024 * half : 1024 * (half + 1)])
        for c in range(8):
            ps = psA.tile([P, TLOC], f32, tag="pj", name=f"qps_{name}_{half}_{c}")
            for kt in range(KT):
                nc.tensor.matmul(ps[:], wt[:, kt, 128 * c : 128 * (c + 1)],
                                 q_rhs[:, kt, :], start=(kt == 0), stop=(kt == KT - 1))
            cg = 8 * half + c
            _cp(nc, cg, QT[:, cg, :], ps[:])

    # ---- K^T projection [2048, S]
    for half in range(2):
        wt = wp.tile([P, KT, 1024], b16, tag="w", name=f"wk_{name}_{half}")
        nc.sync.dma_start(wt[:], wk[:, :, 1024 * half : 1024 * (half + 1)])
        for c in range(8):
            cg = 8 * half + c
            for th in range(S // 512):
                ps = psA.tile([P, 512], f32, tag="pj", name=f"kps_{name}_{cg}_{th}")
                for kt in range(KT):
                    nc.tensor.matmul(ps[:], wt[:, kt, 128 * c : 128 * (c + 1)],
                                     kv_rhs[:, kt, 512 * th : 512 * (th + 1)],
                                     start=(kt == 0), stop=(kt == KT - 1))
                _cp(nc, cg + th, KTt[:, cg, 512 * th : 512 * (th + 1)], ps[:])

    # ---- V projection into ones-augmented [s, (h, 65)] layout
    nc.vector.memset(VA[:, :, :, HS : HS + 1], 1.0)
    wt = wp.tile([P, KT, 1024], b16, tag="w", name=f"wv_{name}")
    nc.sync.dma_start(wt[:], wv)
    for j in range(SJ):
        for c2 in range(2):
            ps = psA.tile([P, 512], f32, tag="pj", name=f"vps_{name}_{j}_{c2}")
            for kt in range(KT):
                nc.tensor.matmul(ps[:], kv_rhs[:, kt, 128 * j : 128 * (j + 1)],
                                 wt[:, kt, 512 * c2 : 512 * (c2 + 1)],
                                 start=(kt == 0), stop=(kt == KT - 1))
            pv = ps.rearrange("p (h d) -> p h d", d=HS)
            _cp(nc, j + c2, VA[:, j, 8 * c2 : 8 * (c2 + 1), 0:HS], pv)

    # ---- per-head stats accumulators
    D1A = stats.tile([H, TLOC], f32, tag="D1A", name=f"D1A_{name}")
    D2A = stats.tile([H, TLOC], f32, tag="D2A", name=f"D2A_{name}")
    ED = stats.tile([H, TLOC], f32, tag="ED", name=f"ED_{name}")
    ssps = psA.tile([H, TLOC], f32, tag="pj", name=f"ss_{name}")
    # ---- per head-pair: scores (both sets packed in PE rows) -> exp -> mask
    #      -> A@V; d-rows staged on partition 0 then DMA-scattered to [H,TLOC]
    #      (compute engines cannot write to arbitrary start partitions).
    #      Stats+combine run per 8-head half so they overlap the other half's
    #      score/AV loops instead of serializing after all 16 heads.
    for k in range(H // 2):
        ds1 = stats.tile([1, 2, TLOC], f32, tag="Ds1", name=f"Ds1_{name}_{k}")
        ds2 = stats.tile([1, 2, TLOC], f32, tag="Ds2", name=f"Ds2_{name}_{k}")
        for hh in range(2):
            h = 2 * k + hh
            o1 = psO.tile([HS + 1, TLOC], f32, tag="o1", name=f"o1_{name}_{h}")
            o2 = psO.tile([HS + 1, TLOC], f32, tag="o2", name=f"o2_{name}_{h}")
            for j in range(SJ):
                ks = slice(128 * j, 128 * (j + 1))
                # both score sets into one 2-bank psum tile -> single wide exp
                ps12 = psS.tile([P, 2 * TLOC], f32, tag="sc", name=f"sc_{name}_{h}_{j}")
                nc.tensor.matmul(ps12[:, 0:TLOC], KTt[0:64, h, ks], QT[0:64, h, :],
                                 start=True, stop=True)
                nc.tensor.matmul(ps12[:, TLOC : 2 * TLOC], KTt[64:128, h, ks],
                                 QT[64:128, h, :], start=True, stop=True)
                e12 = ep.tile([P, 2 * TLOC], b16, tag="e", bufs=3, name=f"e_{name}_{h}_{j}")
                nc.scalar.activation(e12[:], ps12[:], AF.Exp, scale=SCALE)
                if mask is not None:
                    nc.vector.tensor_mul(e12[:, 0:TLOC], e12[:, 0:TLOC], mask[:, j, :])
                    nc.vector.tensor_mul(e12[:, TLOC : 2 * TLOC], e12[:, TLOC : 2 * TLOC],
                                         mask[:, j, :])
                nc.tensor.matmul(o1[:], VA[:, j, h, :], e12[:, 0:TLOC],
                                 start=(j == 0), stop=(j == SJ - 1))
                nc.tensor.matmul(o2[:], VA[:, j, h, :], e12[:, TLOC : 2 * TLOC],
                                 start=(j == 0), stop=(j == SJ - 1))
            r0 = 64 * hh
            nc.vector.tensor_copy(ds1[0:1, hh, :], o1[HS : HS + 1, :])
            nc.vector.tensor_copy(ds2[0:1, hh, :], o2[HS : HS + 1, :])
            nc.vector.tensor_copy(O1S[r0 : r0 + 64, k, :], o1[0:HS, :])
            nc.vector.tensor_copy(O2S[r0 : r0 + 64, k, :], o2[0:HS, :])
        nc.sync.dma_start(D1A[2 * k : 2 * k + 2, :], ds1[:])
        nc.sync.dma_start(D2A[2 * k : 2 * k + 2, :], ds2[:])

    # ---- batched stats + combine
    # ED = eps*d1^2 (before D1A is overwritten); then c = (d1*lam)/d2 in place.
    nc.vector.scalar_tensor_tensor(ED[:], D1A[:], EPS, D1A[:], op0=ALU.mult, op1=ALU.mult)
    nc.vector.reciprocal(D2A[:], D2A[:])
    nc.vector.scalar_tensor_tensor(D1A[:], D1A[:], lam[:], D2A[:], op0=ALU.mult, op1=ALU.mult)
    for k in range(H // 2):
        cb = psS.tile([P, TLOC], f32, tag="sc", name=f"cb_{name}_{k}")
        nc.tensor.matmul(cb[:], patP[:, k, :], D1A[:], start=True, stop=True)
        t1 = ep.tile([P, TLOC], f32, tag="tf", bufs=1, name=f"t1_{name}_{k}")
        nc.vector.tensor_mul(t1[:], O2S[:, k, :], cb[:])
        nc.vector.tensor_sub(O1S[:, k, :], O1S[:, k, :], t1[:])  # u
        us = ep.tile([P, TLOC], b16, tag="us", bufs=2, name=f"us_{name}_{k}")
        nc.vector.tensor_mul(us[:], O1S[:, k, :], O1S[:, k, :])
        nc.tensor.matmul(ssps[:], patB[:, k, :], us[:], start=(k == 0), stop=(k == H // 2 - 1))

    # r = (1-lam0) * rsqrt(ss/HS + eps*d1^2), via exp(-0.5*ln(v))
    nc.vector.scalar_tensor_tensor(ED[:], ssps[:], 1.0 / HS, ED[:], op0=ALU.mult, op1=ALU.add)
    nc.scalar.activation(ED[:], ED[:], AF.Ln)
    nc.scalar.activation(ED[:], ED[:], AF.Exp, scale=-0.5)
    nc.vector.tensor_scalar_mul(ED[:], ED[:], 1.0 - LAM0)
    r_all = ED

    for k in range(H // 2):
        rb = psS.tile([P, TLOC], f32, tag="sc", name=f"rb_{name}_{k}")
        nc.tensor.matmul(rb[:], patP[:, k, :], r_all[:], start=True, stop=True)
        nc.vector.tensor_mul(QT[:, k, :], O1S[:, k, :], rb[:])  # o_norm
    ONS = QT  # scores are done; reuse the first 8 QT chunks as o_norm storage

    # ---- Wo projection + residual / doubling
    wt = wp.tile([P, KT, 1024], b16, tag="w", name=f"wo_{name}")
    nc.sync.dma_start(wt[:], wo)
    for c in range(KT):
        ps = psA.tile([P, TLOC], f32, tag="pj", name=f"ops_{name}_{c}")
        for kk in range(8):
            nc.tensor.matmul(ps[:], wt[:, kk, 128 * c : 128 * (c + 1)], ONS[:, kk, :],
                             start=(kk == 0), stop=(kk == 7))
        if resid is not None:
            nc.vector.tensor_add(out_ap[:, c, :], ps[:], resid[:, c, :])
        else:
            nc.scalar.mul(out_ap[:, c, :], ps[:], 2.0)


def build_program(sim_compat=False, reps=1):
    # sim_compat: CoreSim lacks Silu; emit Sigmoid + explicit multiply instead
    # (identical math) so the program can be numerically validated in sim.
    nc = bacc.Bacc("TRN2", target_bir_lowering=False, debug=False)

    dt = nc.dram_tensor
    xT = dt("xT", [P, KT, T], f32, kind="ExternalInput").ap()
    encT = dt("encT", [P, KT, T], b16, kind="ExternalInput").ap()
    wq_s = dt("wq_s", [P, KT, 2048], b16, kind="ExternalInput").ap()
    wk_s = dt("wk_s", [P, KT, 2048], b16, kind="ExternalInput").ap()
    wv_s = dt("wv_s", [P, KT, 1024], b16, kind="ExternalInput").ap()
    wo_s = dt("wo_s", [P, KT, 1024], b16, kind="ExternalInput").ap()
    wq_c = dt("wq_c", [P, KT, 2048], b16, kind="ExternalInput").ap()
    wk_c = dt("wk_c", [P, KT, 2048], b16, kind="ExternalInput").ap()
    wv_c = dt("wv_c", [P, KT, 1024], b16, kind="ExternalInput").ap()
    wo_c = dt("wo_c", [P, KT, 1024], b16, kind="ExternalInput").ap()
    w1 = dt("w1", [P, KT, DFF], b16, kind="ExternalInput").ap()
    w2 = dt("w2", [P, KT, DFF], b16, kind="ExternalInput").ap()
    w3 = dt("w3", [P, FFC, D], b16, kind="ExternalInput").ap()
    g = dt("g", [P, KT], f32, kind="ExternalInput").ap()
    maskT = dt("maskT", [P, SJ, TLOC], b16, kind="ExternalInput").ap()
    lqk = {n: dt(n, [H, HS], f32, kind="ExternalInput").ap()
           for n in ("lq1_s", "lk1_s", "lq2_s", "lk2_s",
                     "lq1_c", "lk1_c", "lq2_c", "lk2_c")}
    patP_d = dt("patP", [H, H // 2, P], f32, kind="ExternalInput").ap()
    patB_d = dt("patB", [P, H // 2, H], b16, kind="ExternalInput").ap()
    out_d = dt("out", [P, KT, TLOC], f32, kind="ExternalOutput").ap()

    with tile.TileContext(nc) as tc:
        with contextlib.ExitStack() as _loop:
            if reps > 1:
                # benchmark mode: run the whole kernel `reps` times inside one
                # NEFF so device time dominates the axon dispatch quantum
                _loop.enter_context(tc.For_i(0, reps, 1))
            with contextlib.ExitStack() as top:
                constp = top.enter_context(tc.tile_pool(name="const", bufs=1))
                persist = top.enter_context(tc.tile_pool(name="persist", bufs=1))

                gS = constp.tile([P, KT], f32, tag="g", name="gS")
                nc.sync.dma_start(gS[:], g)
                ones_c = constp.tile([P, 1], f32, tag="ones_c", name="ones_c")
                nc.vector.memset(ones_c[:], 1.0)
                ones_r = constp.tile([1, P], f32, tag="ones_r", name="ones_r")
                nc.vector.memset(ones_r[:], 1.0)

                H1b = persist.tile([P, KT, TLOC], b16, tag="H1b", name="H1b")
                H2 = persist.tile([P, KT, TLOC], f32, tag="H2", name="H2")

                # shared pools for stages 0-2 (both attention blocks + rmsnorm0):
                # weight slots / exp tiles / stats / the full 8-bank PSUM budget
                s012 = top.enter_context(contextlib.ExitStack())
                shared = {
                    "wp": s012.enter_context(tc.tile_pool(name="wp", bufs=2)),
                    "ep": s012.enter_context(tc.tile_pool(name="ep", bufs=4)),
                    "stats": s012.enter_context(tc.tile_pool(name="stats", bufs=1)),
                    "psA": s012.enter_context(tc.tile_pool(name="psA", bufs=2, space="PSUM")),
                    "psS": s012.enter_context(tc.tile_pool(name="psS", bufs=2, space="PSUM")),
                    "psO": s012.enter_context(tc.tile_pool(name="psO", bufs=1, space="PSUM")),
                }

                # ---- stage 0+1: rmsnorm(x) -> self-attention (+residual)
                with contextlib.ExitStack() as s01:
                    s01p = s01.enter_context(tc.tile_pool(name="s01", bufs=1))
                    hT = s01p.tile([P, KT, T], b16, tag="hT", name="hT")
                    maskS = s01p.tile([P, SJ, TLOC], b16, tag="mask", name="maskS")
                    with contextlib.ExitStack() as s0:
                        xp = s0.enter_context(tc.tile_pool(name="s0x", bufs=1))
                        xS = xp.tile([P, KT, T], f32, tag="xT", name="xS")
                        nc.sync.dma_start(xS[:, :, 0:TLOC], xT[:, :, 0:TLOC])
                        nc.sync.dma_start(xS[:, :, TLOC:T], xT[:, :, TLOC:T])
                        _rmsnorm(nc, tc, s0, xS, gS, ones_c, ones_r, hT, T, "n0",
                                 psp=shared["psA"])
                    # consts needed later; emitted off the startup-critical path
                    nc.sync.dma_start(maskS[:], maskT)
                    patP = constp.tile([H, H // 2, P], f32, tag="patP", name="patP_s")
                    nc.sync.dma_start(patP[:], patP_d)
                    patB = constp.tile([P, H // 2, H], b16, tag="patB", name="patB_s")
                    nc.sync.dma_start(patB[:], patB_d)
                    lq = {}
                    for n, ap in lqk.items():
                        t = constp.tile([H, HS], f32, tag=n, name=f"{n}_s")
                        nc.sync.dma_start(t[:], ap)
                        lq[n] = t
                    lam_s = _lam_from_inputs(nc, constp, lq["lq1_s"], lq["lk1_s"],
                                             lq["lq2_s"], lq["lk2_s"], "s")
                    lam_c = _lam_from_inputs(nc, constp, lq["lq1_c"], lq["lk1_c"],
                                             lq["lq2_c"], lq["lk2_c"], "c")
                    with contextlib.ExitStack() as s1:
                        _attn(nc, tc, s1, shared, q_rhs=hT[:, :, 0:TLOC], kv_rhs=hT,
                              wq=wq_s, wk=wk_s, wv=wv_s, wo=wo_s, lam=lam_s,
                              mask=maskS, patP=patP, patB=patB,
                              resid=hT[:, :, 0:TLOC], out_ap=H1b, name="s")

                # ---- stage 2: cross-attention, h2 = 2*attn
                with contextlib.ExitStack() as s2:
                    s2p = s2.enter_context(tc.tile_pool(name="s2", bufs=1))
                    eS = s2p.tile([P, KT, T], b16, tag="encT", name="eS")
                    nc.sync.dma_start(eS[:], encT)
                    _attn(nc, tc, s2, shared, q_rhs=H1b, kv_rhs=eS,
                          wq=wq_c, wk=wk_c, wv=wv_c, wo=wo_c, lam=lam_c,
                          mask=None, patP=patP, patB=patB,
                          resid=None, out_ap=H2, name="c")

                s012.close()

                # ---- stage 3+4: rmsnorm(h2) -> SwiGLU + h2
                with contextlib.ExitStack() as s34:
                    s34p = s34.enter_context(tc.tile_pool(name="s34", bufs=1))
                    H3b = s34p.tile([P, KT, TLOC], b16, tag="H3b", name="H3b")
                    AFt = s34p.tile([P, FFC, TLOC], b16, tag="AF", name="AFt")
                    _rmsnorm(nc, tc, s34, H2, gS, ones_c, ones_r, H3b, TLOC, "n2")
                    wp = s34.enter_context(tc.tile_pool(name="ffw", bufs=2))
                    w3p = s34.enter_context(tc.tile_pool(name="ffw3", bufs=1))
                    psp = s34.enter_context(tc.tile_pool(name="ffps", bufs=4, space="PSUM"))
                    sp = s34.enter_context(tc.tile_pool(name="ffs", bufs=3))
                    outp = s34.enter_context(tc.tile_pool(name="ffo", bufs=2))
                    for q in range(8):  # DFF in 8 eighths of 4 chunks
                        wt1 = wp.tile([P, KT, 512], b16, tag="fw", bufs=4, name=f"w1_{q}")
                        nc.sync.dma_start(wt1[:], w1[:, :, 512 * q : 512 * (q + 1)])
                        wt2 = wp.tile([P, KT, 512], b16, tag="fw", bufs=4, name=f"w2_{q}")
                        nc.sync.dma_start(wt2[:], w2[:, :, 512 * q : 512 * (q + 1)])
                        for c in range(4):
                            f = 4 * q + c
                            ps1 = psp.tile([P, TLOC], f32, tag="f1", name=f"p1_{f}")
                            for kt in range(KT):
                                nc.tensor.matmul(ps1[:], wt1[:, kt, 128 * c : 128 * (c + 1)],
                                                 H3b[:, kt, :], start=(kt == 0), stop=(kt == KT - 1))
                            s1t = sp.tile([P, TLOC], b16, tag="s1", name=f"s1_{f}")
                            if sim_compat:
                                nc.scalar.activation(s1t[:], ps1[:], AF.Sigmoid)
                                nc.vector.tensor_mul(s1t[:], s1t[:], ps1[:])
                            else:
                                nc.scalar.activation(s1t[:], ps1[:], AF.Silu)
                            ps2 = psp.tile([P, TLOC], f32, tag="f1", name=f"p2_{f}")
                            for kt in range(KT):
                                nc.tensor.matmul(ps2[:], wt2[:, kt, 128 * c : 128 * (c + 1)],
                                                 H3b[:, kt, :], start=(kt == 0), stop=(kt == KT - 1))
                            nc.vector.tensor_mul(AFt[:, f, :], s1t[:], ps2[:])
                    # W3: full DFF contraction per output chunk, whole W3 resident
                    wt3 = w3p.tile([P, FFC, D], b16, tag="w3", name="w3S")
                    nc.sync.dma_start(wt3[:], w3)
                    for c in range(KT):
                        ps = psp.tile([P, TLOC], f32, tag="f1", name=f"p3_{c}")
                        for ff in range(FFC):
                            nc.tensor.matmul(ps[:], wt3[:, ff, 128 * c : 128 * (c + 1)],
                                             AFt[:, ff, :], start=(ff == 0), stop=(ff == FFC - 1))
                        ot = outp.tile([P, TLOC], f32, tag="o", name=f"out_{c}")
                        nc.vector.tensor_add(ot[:], ps[:], H2[:, c, :])
                        nc.sync.dma_start(out_d[:, c, :], ot[:])

    nc.compile()
    return nc


# ============================================================= host glue ==

def _dev3(a, p=P):
    """[N*p, W] -> [p, N, W] device layout (partition-inner)."""
    n, w = a.shape[0] // p, a.shape[1]
    return np.ascontiguousarray(a.reshape(n, p, w).transpose(1, 0, 2))


def prep_core_inputs(inputs, core):
    b, half = core // 2, core % 2
    f4 = lambda a: np.asarray(a, dtype=np.float32)
    perm = np.arange(T)
    if half == 1:
        perm = np.concatenate([np.arange(TLOC, T), np.arange(0, TLOC)])

    xt = f4(inputs["x"][b]).T[:, perm]            # [D, T] permuted cols
    et = f4(inputs["encoder_output"][b]).T        # [D, T]

    ot = perm[:TLOC]                              # original index of local tokens
    mask = (perm[:, None] <= ot[None, :])         # [S, TLOC] key(perm) <= query(orig)
    maskT = _dev3(mask.astype(bf))

    m = {
        "xT": _dev3(xt.astype(np.float32)),
        "encT": _dev3(et.astype(bf)),
        "maskT": maskT,
    }
    for n_dev, n_in in (("wq_s", "Wq_s"), ("wk_s", "Wk_s"), ("wv_s", "Wv_s"),
                        ("wo_s", "Wo_s"), ("wq_c", "Wq_c"), ("wk_c", "Wk_c"),
                        ("wv_c", "Wv_c"), ("wo_c", "Wo_c"),
                        ("w1", "W1"), ("w2", "W2"), ("w3", "W3")):
        m[n_dev] = _dev3(f4(inputs[n_in]).astype(bf))
    m["g"] = np.ascontiguousarray(f4(inputs["g_rms"]).reshape(KT, P).T)
    for nm in ("lq1_s", "lk1_s", "lq2_s", "lk2_s", "lq1_c", "lk1_c", "lq2_c", "lk2_c"):
        m[nm] = f4(inputs[nm])
    pp = np.zeros((H, H // 2, P), np.float32)
    pb = np.zeros((P, H // 2, H), np.float32)
    for k in range(H // 2):
        for p in range(P):
            i = 2 * k + (1 if p >= 64 else 0)
            pp[i, k, p] = 1.0
            pb[p, k, i] = 1.0
    m["patP"] = pp
    m["patB"] = pb.astype(bf)
    return m


_SHARED = ("wq_s", "wk_s", "wv_s", "wo_s", "wq_c", "wk_c", "wv_c", "wo_c",
           "w1", "w2", "w3", "g", "patP", "patB",
           "lq1_s", "lk1_s", "lq2_s", "lk2_s", "lq1_c", "lk1_c", "lq2_c", "lk2_c")


def prep_all_inputs(inputs):
    maps = []
    shared = None
    for core in range(N_CORES):
        m = prep_core_inputs(inputs, core)
        if shared is None:
            shared = {k: m[k] for k in _SHARED}
        else:
            for k in _SHARED:
                m[k] = shared[k]
        maps.append(m)
    return maps


def assemble_output(results):
    out = np.empty((B, T, D), np.float32)
    for core in range(N_CORES):
        b, half = core // 2, core % 2
        o = results[core]["out"]  # [P, KT, TLOC]
        out[b, TLOC * half : TLOC * (half + 1), :] = (
            o.transpose(2, 1, 0).reshape(TLOC, D))
    return out


_NC_CACHE = {}


def _get_program():
    if "nc" not in _NC_CACHE:
        _NC_CACHE["nc"] = build_program()
    return _NC_CACHE["nc"]


def run(inputs, trace=False):
    from concourse.bass_utils import run_bass_kernel_spmd
    nc = _get_program()
    in_maps = prep_all_inputs(inputs)
    res = run_bass_kernel_spmd(nc, in_maps, core_ids=list(range(N_CORES)), trace=trace)
    return assemble_output(res.results), res


def kernel(**inputs):
    out, _ = run(inputs)
    return out

